# revision 1
# baseline (speedup 1.0000x reference)
"""Causal self-attention (B=4, T=2048, C=1024, H=16, D=64) on 8 trn2 cores.

Sharding: core i handles batch b = i//2 and head-group g = i%2 (8 of 16
heads), tensor-parallel over c_attn columns / c_proj rows. Each core
computes qkv for its heads, causal attention, and a partial projection
(its 512 rows of w_proj); the host sums the two partials per batch and
adds b_proj.

Per-core device pipeline (feature-major layouts avoid all transposes):
  xT [C, T] (host-transposed; bf16 for q/k via in-flight DMA cast, f32r
  for the v path)  --matmul-->  qkT [1024, T] feature-major (bf16)
                   --matmul-->  v [T, 8*(64+1)] token-major (+ones cols)
  S^T strips [j, i] = kT_h.T @ qT_h (K=64, fp32 psum), exp on ACT with
  scale=1/8, strips trimmed to the exact causal start (jc*128); one
  [128,128] upper-tri mask multiply on the diagonal block (widened to
  256 cols on jc%4==3 strips so every matmul has N>=256),
  yT_aug [65, i] += v_aug.T @ expS (row 64 = softmax denominators, via
  the ones column), normalize with reciprocal + gpsimd
  partition_broadcast, out[t, o] = sum_l yT[l, t].T @ w_p[l, o].

Scheduling: emission order is per-engine execution order, so attention
strips (ACT-heavy) are interleaved with qkv/projection matmul groups
(PE-heavy) through a demand-driven filler queue; attention head-passes
start as soon as their qk feature chunks land. PSUM: psA 2 + psS 2x2 +
psY 2 = 8 banks.
"""

import sys

sys.path.insert(0, "/opt/trn_rl_repo")

from collections import deque
from contextlib import ExitStack

import ml_dtypes
import numpy as np

import concourse.bass as bass
import concourse.mybir as mybir
import concourse.tile as tile
from concourse import bacc
from concourse import bass_utils

f32 = mybir.dt.float32
f32r = mybir.dt.float32r
bf16 = mybir.dt.bfloat16
EXP = mybir.ActivationFunctionType.Exp
MUL = mybir.AluOpType.mult
ADD = mybir.AluOpType.add

B, T, C, H, D = 4, 2048, 1024, 16, 64
HL = H // 2          # 8 heads per core
CL = HL * D          # 512 local feature width
P = 128
KC = C // P          # 8 contraction chunks over C
NJC = T // P         # 16 token chunks of 128
NIC = T // 512       # 4 i-chunks of 512
NTC = T // 512       # 4 t-chunks of 512 in phase A
JQK = 2 * CL // P    # 8 qk feature chunks of 128


def build_body(tc, aps):
    nc = tc.nc
    wqk, bqk, wv, bv, wp, masks, outp = (
        aps["wqk"], aps["bqk"], aps["wv"], aps["bv"],
        aps["wp"], aps["masks"], aps["outp"],
    )

    with ExitStack() as ctx:
        const = ctx.enter_context(tc.tile_pool(name="const", bufs=1))
        qkT_pool = ctx.enter_context(tc.tile_pool(name="qkT", bufs=1))
        vaug_pool = ctx.enter_context(tc.tile_pool(name="vaug", bufs=1))
        yT_pool = ctx.enter_context(tc.tile_pool(name="yT", bufs=1))

        masks_sb = const.tile([P, P], f32)
        nc.sync.dma_start(masks_sb[:], masks[:])
        mask2_sb = const.tile([P, 2 * P], f32)
        nc.sync.dma_start(mask2_sb[:], aps["mask2"][:])
        bqk_sb = const.tile([P, JQK], f32)
        nc.sync.dma_start(bqk_sb[:], bqk.rearrange("(j p) -> p j", p=P))
        bv_rep = const.tile([P, CL], f32)
        nc.sync.dma_start(bv_rep[:], bv[None, :].to_broadcast([P, CL]))

        qkT = qkT_pool.tile([P, JQK, T], bf16)
        vaug = vaug_pool.tile([P, NJC, HL, D + 1], f32r)
        nc.vector.memset(
            vaug[:, :, :, D : D + 1].bitcast(mybir.dt.uint32), 0x3F800000
        )

        # ---------------- Phase A/B interleaved.
        # A1 builds v + qk for head-pairs 0,1; B(0..3) (ACT-heavy) then
        # overlaps A2 (PE-heavy qk for pairs 2,3, re-streaming xT);
        # B(4..7) follows. PSUM: psA 2 + psS 4 + psY 2 = 8 banks.
        def qk_chunk_thunks(j, xT_sb):
            # one thunk per (chunk, tci) psum group; first thunk loads wq
            wq_box = {}

            def load(tci):
                if tci == 0:
                    wq_t = wq_pool.tile([P, KC, P], bf16, tag="wq", name=f"wq{j}")
                    nc.sync.dma_start(wq_t[:], wqk[j])
                    wq_box["t"] = wq_t
                wq_t = wq_box["t"]
                ps = psA.tile([P, 512], f32, tag="a")
                for k in range(KC):
                    nc.tensor.matmul(
                        ps[:], wq_t[:, k, :],
                        xT_sb[:, k, tci * 512 : tci * 512 + 512],
                        start=(k == 0), stop=(k == KC - 1),
                    )
                nc.scalar.activation(
                    qkT[:, j, tci * 512 : tci * 512 + 512], ps[:],
                    mybir.ActivationFunctionType.Identity,
                    bias=bqk_sb[:, j : j + 1],
                )

            return [lambda tci=tci: load(tci) for tci in range(NTC)]

        def v_chunk_thunks(xT_sb, wv_sb):
            xTv_r = aps["xTv"].rearrange("(k p) t -> p k t", p=P)

            def vchunk(tci128):
                xtv = xtv_pool.tile([P, KC, P], f32r, tag="xtv")
                nc.sync.dma_start(
                    xtv[:], xTv_r[:, :, tci128 * P : (tci128 + 1) * P]
                )
                ps = psA.tile([P, 512], f32, tag="a")
                for k in range(KC):
                    nc.tensor.matmul(
                        ps[:],
                        xtv[:, k, :],
                        wv_sb[:, k, :],
                        start=(k == 0), stop=(k == KC - 1),
                    )
                nc.vector.tensor_tensor(
                    vaug[:, tci128, :, 0:D],
                    ps[:].rearrange("p (h d) -> p h d", h=HL),
                    bv_rep[:].rearrange("p (h d) -> p h d", h=HL), ADD,
                )

            return [lambda t=t: vchunk(t) for t in range(NJC)]

        def emit_b_pass(h, pp, drain):
            pr, half = h // 2, h % 2
            base = half * 64
            qTh = qkT[base : base + 64, pr, :]
            kTh = qkT[base : base + 64, 4 + pr, :]

            if True:
                lo = pp * 1024
                yt_ps = [
                    psY.tile([P, 512], f32, tag="yt", name=f"yt{h}_{pp}_{i}")
                    for i in range(2)
                ]
                for jc in range(8 * pp + 8):
                    if h < 2:
                        need(jc + 1)
                    else:
                        need(16 + 8 * (h // 2))
                        if pp == 1 and h >= 2:
                            # spread the first-half projection tiles across
                            # the late ACT-bound head passes
                            need(40 + min(16, (h - 2) * 3 + jc // 6))
                    drain()
                    diag = jc * P >= lo
                    # widen width-128 diagonal strips to 256 (fp32r runs at
                    # 4 cyc/row below N=256); the extra 128 cols are zeroed
                    # by the wider mask.
                    wide = diag and jc % 4 == 3
                    start = max(jc * P - (P if wide else 0), lo)
                    ps_s = psS.tile([P, 1024], f32, tag="s")
                    for icp in range(2):
                        ic = 2 * pp + icp
                        if (ic + 1) * 512 <= start:
                            continue
                        c0 = max(start, ic * 512)
                        nc.tensor.matmul(
                            ps_s[:, c0 - lo : (icp + 1) * 512],
                            kTh[:, jc * P : (jc + 1) * P],
                            qTh[:, c0 : (ic + 1) * 512],
                            start=True, stop=True,
                        )
                    w = lo + 1024 - start
                    es = es_pool.tile([P, 1024], f32r, tag="es")
                    nc.scalar.activation(
                        es[:, :w], ps_s[:, start - lo : 1024], EXP, scale=0.125
                    )
                    if wide:
                        nc.vector.tensor_tensor(
                            es[:, 0 : 2 * P], es[:, 0 : 2 * P], mask2_sb[:], MUL
                        )
                    elif diag:
                        nc.vector.tensor_tensor(
                            es[:, 0:P], es[:, 0:P], masks_sb[:], MUL
                        )
                    for icp in range(2):
                        ic = 2 * pp + icp
                        if (ic + 1) * 512 <= start:
                            continue
                        c0 = max(start, ic * 512)
                        nc.tensor.matmul(
                            yt_ps[icp][0 : D + 1, c0 - ic * 512 : 512],
                            vaug[:, jc, h, :],
                            es[:, c0 - start : c0 - start + 512 - (c0 - ic * 512)],
                            start=(jc == 0), stop=(jc == 4 * ic + 3),
                        )
                for icp in range(2):
                    ic = 2 * pp + icp
                    rec = nrm_pool.tile([1, 512], f32, tag="rec")
                    nc.vector.reciprocal(rec[0:1, :], yt_ps[icp][D : D + 1, :])
                    rep = nrm_pool.tile([64, 512], f32, tag="rep")
                    nc.gpsimd.partition_broadcast(rep[:], rec[0:1, :])
                    nc.vector.tensor_tensor(
                        yT[base : base + 64, pr, ic * 512 : ic * 512 + 512],
                        yt_ps[icp][0:64, :], rep[:], MUL,
                    )

        yT = yT_pool.tile([P, CL // P, T], f32r)
        with ExitStack() as actx:
            xt_pool = actx.enter_context(tc.tile_pool(name="xt", bufs=1))
            xtv_pool = actx.enter_context(tc.tile_pool(name="xtv", bufs=3))
            wq_pool = actx.enter_context(tc.tile_pool(name="wq", bufs=2))
            psA = actx.enter_context(tc.tile_pool(name="psA", bufs=2, space="PSUM"))

            filler = deque()
            _dr = {"n": 0, "num": 0, "den": 1, "done": 0}

            def drain():
                # pop fillers at num/den per strip so PE-side filler work
                # spreads into the ACT-bound stretch instead of front-loading
                _dr["n"] += _dr["num"]
                while filler and _dr["n"] >= _dr["den"]:
                    _dr["n"] -= _dr["den"]
                    filler.popleft()()
                    _dr["done"] += 1

            def need(k):
                # force-drain so producers (v chunks, qk chunks) are emitted
                # before the strips that read them
                while filler and _dr["done"] < k:
                    filler.popleft()()
                    _dr["done"] += 1

            with tc.tile_pool(name="wv", bufs=1) as wv_pool:
                xT_sb = xt_pool.tile([P, KC, T], bf16)
                xT_r = aps["xTv"].rearrange("(k p) t -> p k t", p=P)

                def xq_dma(q):
                    # gpsimd DMA casts f32 -> bf16 in flight
                    nc.gpsimd.dma_start(
                        xT_sb[:, :, q * 512 : (q + 1) * 512],
                        xT_r[:, :, q * 512 : (q + 1) * 512],
                    )

                wv_sb = wv_pool.tile([P, KC, CL], f32r)

                # pair-0 qk chunks emitted up front, interleaved with the
                # xT quarter loads so the PE starts ~5us in; everything
                # else goes through the filler queue between B strips.
                xq_dma(0)
                q0 = qk_chunk_thunks(0, xT_sb)
                q4 = qk_chunk_thunks(4, xT_sb)
                q0[0]()
                q4[0]()
                for q in (1, 2, 3):
                    xq_dma(q)
                    q0[q]()
                    q4[q]()
                nc.sync.dma_start(
                    wv_sb[:], wv.rearrange("(k p) n -> p k n", p=P)
                )
                filler.extend(v_chunk_thunks(xT_sb, wv_sb))
                for j in (1, 5):
                    filler.extend(qk_chunk_thunks(j, xT_sb))

                with ExitStack() as bctx:
                    es_pool = bctx.enter_context(tc.tile_pool(name="es", bufs=3))
                    nrm_pool = bctx.enter_context(tc.tile_pool(name="nrm", bufs=1))
                    ostg = bctx.enter_context(tc.tile_pool(name="ostg", bufs=4))
                    wp_pool = bctx.enter_context(tc.tile_pool(name="wp", bufs=1))
                    psS = bctx.enter_context(
                        tc.tile_pool(name="psS", bufs=2, space="PSUM")
                    )
                    psY = bctx.enter_context(
                        tc.tile_pool(name="psY", bufs=2, space="PSUM")
                    )

                    def c_tile(tcb, oc):
                        ps = psA.tile([P, 512], f32, tag="a")
                        for li in range(4):
                            nc.tensor.matmul(
                                ps[:],
                                yT[:, li, tcb * P : (tcb + 1) * P],
                                wp_sb[:, li, oc * 512 : oc * 512 + 512],
                                start=(li == 0), stop=(li == 3),
                            )
                        ot = ostg.tile([P, 512], f32, tag="o")
                        if tcb >= 8 or (tcb + oc) % 2 == 0:
                            nc.scalar.copy(ot[:], ps[:])
                        else:
                            nc.vector.tensor_copy(ot[:], ps[:])
                        nc.sync.dma_start(
                            outp[tcb * P : (tcb + 1) * P,
                                 oc * 512 : oc * 512 + 512],
                            ot[:],
                        )

                    emit_b_pass(0, 0, drain)
                    for j in (2, 6, 3, 7):
                        filler.extend(qk_chunk_thunks(j, xT_sb))
                    emit_b_pass(0, 1, drain)
                    emit_b_pass(1, 0, drain)
                    emit_b_pass(1, 1, drain)
                    emit_b_pass(2, 0, drain)
                    emit_b_pass(3, 0, drain)
                    emit_b_pass(4, 0, drain)
                    wp_sb = wp_pool.tile([P, 4, C], f32r, name="wp_sb")
                    nc.sync.dma_start(
                        wp_sb[:], wp.rearrange("(l p) n -> p l n", p=P)
                    )
                    emit_b_pass(5, 0, drain)
                    emit_b_pass(6, 0, drain)
                    emit_b_pass(7, 0, drain)
                    filler.extend(
                        lambda t=t, o=o: c_tile(t, o)
                        for t in range(8) for o in range(2)
                    )
                    for h in range(2, 8):
                        emit_b_pass(h, 1, drain)
                    while filler:
                        filler.popleft()()
                    for t in range(8, NJC):
                        for o in range(2):
                            c_tile(t, o)


_CACHE = {}


def build_nc():
    if "nc" in _CACHE:
        return _CACHE["nc"]
    nc = bacc.Bacc(
        "TRN2",
        target_bir_lowering=False,
        debug=False,
        enable_asserts=False,
        num_devices=8,
    )
    aps = {
        "wqk": nc.dram_tensor("wqk", [JQK, P, KC, P], bf16, kind="ExternalInput").ap(),
        "bqk": nc.dram_tensor("bqk", [2 * CL], f32, kind="ExternalInput").ap(),
        "wv": nc.dram_tensor("wv", [C, CL], f32r, kind="ExternalInput").ap(),
        "xTv": nc.dram_tensor("xTv", [C, T], f32r, kind="ExternalInput").ap(),
        "bv": nc.dram_tensor("bv", [CL], f32, kind="ExternalInput").ap(),
        "wp": nc.dram_tensor("wp", [CL, C], f32r, kind="ExternalInput").ap(),
        "masks": nc.dram_tensor("masks", [P, P], f32, kind="ExternalInput").ap(),
        "mask2": nc.dram_tensor("mask2", [P, 2 * P], f32, kind="ExternalInput").ap(),
        "outp": nc.dram_tensor("outp", [T, C], f32, kind="ExternalOutput").ap(),
    }
    with tile.TileContext(nc) as tc:
        build_body(tc, aps)
    nc.compile()
    _CACHE["nc"] = nc
    return nc


def make_masks():
    # mask[jp, c] = 1 where column c (global i = jc*128 + c) >= row jp (j)
    return np.triu(np.ones((P, P), dtype=np.float32))


def make_mask2():
    m = np.zeros((P, 2 * P), dtype=np.float32)
    m[:, P:] = np.triu(np.ones((P, P), dtype=np.float32))
    return m


def make_in_maps(x, w_attn, b_attn, w_proj, b_proj):
    masks = make_masks()
    mask2 = make_mask2()
    in_maps = []
    xTs = [np.ascontiguousarray(x[b].T) for b in range(B)]
    for core in range(8):
        b, g = core // 2, core % 2
        xT_f32 = xTs[b]
        qcols = slice(g * CL, (g + 1) * CL)
        kcols = slice(C + g * CL, C + (g + 1) * CL)
        vcols = slice(2 * C + g * CL, 2 * C + (g + 1) * CL)
        in_maps.append(
            {
                "wqk": np.ascontiguousarray(
                    np.concatenate([w_attn[:, qcols], w_attn[:, kcols]], axis=1)
                    .astype(ml_dtypes.bfloat16)
                    .reshape(KC, P, JQK, P)
                    .transpose(2, 1, 0, 3)
                ),
                "bqk": np.ascontiguousarray(
                    np.concatenate([b_attn[qcols], b_attn[kcols]])
                ),
                "wv": np.ascontiguousarray(w_attn[:, vcols]),
                "xTv": xT_f32,
                "bv": np.ascontiguousarray(b_attn[vcols]),
                "wp": np.ascontiguousarray(w_proj[g * CL : (g + 1) * CL, :]),
                "masks": masks,
                "mask2": mask2,
            }
        )
    return in_maps


def combine(parts, b_proj):
    return np.stack(
        [parts[2 * b] + parts[2 * b + 1] + b_proj[None, :] for b in range(B)]
    ).astype(np.float32)


def kernel(x, w_attn, b_attn, w_proj, b_proj, _trace=False, **run_kwargs):
    x = np.asarray(x, dtype=np.float32)
    w_attn = np.asarray(w_attn, dtype=np.float32)
    b_attn = np.asarray(b_attn, dtype=np.float32)
    w_proj = np.asarray(w_proj, dtype=np.float32)
    b_proj = np.asarray(b_proj, dtype=np.float32)

    nc = build_nc()
    in_maps = make_in_maps(x, w_attn, b_attn, w_proj, b_proj)
    try:
        res = bass_utils.run_bass_kernel_spmd(
            nc, in_maps, core_ids=list(range(8)), trace=_trace, **run_kwargs
        )
    except Exception:
        # transient NRT device wedge: one retry
        res = bass_utils.run_bass_kernel_spmd(
            nc, in_maps, core_ids=list(range(8)), trace=_trace, **run_kwargs
        )
    parts = [res.results[i]["outp"] for i in range(8)]
    out = combine(parts, b_proj)
    if _trace:
        return out, res
    return out



# revision 15
# speedup vs baseline: 1.0916x; 1.0916x over previous
"""Causal self-attention (B=4, T=2048, C=1024, H=16, D=64) on 8 trn2 cores.

Sharding: core i handles batch b = i//2 and head-group g = i%2 (8 of 16
heads), tensor-parallel over c_attn columns / c_proj rows. Each core
computes qkv for its heads, causal attention, and a partial projection
(its 512 rows of w_proj); the host sums the two partials per batch and
adds b_proj.

Per-core pipeline (all matmuls bf16, 1 cyc/row):
  xT [128, 8k, T] bf16 (host-transposed+cast)
    --matmul--> qkT [128, 8j, T] feature-major (DVE bias copy)
    --matmul--> vaug [128, 16jc, 8h, 64+1] token-major (ones col = denom)
  Attention runs in (head, 512-query block qb) passes. Per strip jc:
  S^T [128 keys, <=512 queries] = kT_h.T @ qT_h (K=64) into a 1-bank psS
  tile, exp on ACT (scale 1/8) -> es bf16, triu mask (DVE) on the
  diagonal 128-block. AV is flipped (keys moving): psY[:, c, :] +=
  es[:, qchunk].T @ vaug[:, jc, h, :] -- 65 rows per (key-block,
  query-chunk) instead of 128+, halving AV PE time. psY is a 1-bank
  [128, 4, 65] tile; sibling chunk groups share the bank, so matmuls use
  skip_group_check with explicit DVE memset zeroing instead of start=True
  (which would zero the whole bank).
  Normalize is batched at pass end (one reciprocal of the 4 denom cols +
  4 per-partition tensor_scalar) -> y_tok bf16, then one DMA-transpose
  (zero PE cost) -> yT [128l, 4li, 16tcb, 128] feature-major, then psY
  memset. Two psY tiles alternate by pass parity so no engine ever reads
  a PSUM bank that PE is still writing (bank collisions are fatal on HW
  and invisible to CoreSim).
  Projection: psA[128t,512] = sum_li yT.T @ wp (bf16), DVE/ACT copy,
  DMA out.

Scheduling: emission order is per-engine execution order. Attention
strips (ACT-exp-bound) are interleaved with qkv/projection matmuls via a
keyed filler queue: need(key) force-drains producers before consumers,
drain(frac) paces the rest into the ACT-bound stretches. AV for strip jc
is emitted after S of strip jc+1 so exp(jc) hides behind PE work.
PSUM banks: psA 2x1 + psS 4x1 + psY 2x1 = 8.
"""

import sys

sys.path.insert(0, "/opt/trn_rl_repo")

from collections import deque
from contextlib import ExitStack

import ml_dtypes
import numpy as np

import concourse.bass as bass
import concourse.mybir as mybir
import concourse.tile as tile
from concourse import bacc
from concourse import bass_utils

f32 = mybir.dt.float32
bf16 = mybir.dt.bfloat16
EXP = mybir.ActivationFunctionType.Exp
MUL = mybir.AluOpType.mult
ADD = mybir.AluOpType.add

B, T, C, H, D = 4, 2048, 1024, 16, 64
HL = H // 2          # 8 heads per core
CL = HL * D          # 512 local feature width
P = 128
KC = C // P          # 8 contraction chunks over C
NJC = T // P         # 16 token chunks of 128
JQK = 2 * CL // P    # 8 qk feature chunks of 128
NQB = T // 512       # 4 query blocks per head


def build_body(tc, aps):
    nc = tc.nc
    wqk, bqk, wv, bv, wp, masks, outp, xT = (
        aps["wqk"], aps["bqk"], aps["wv"], aps["bv"],
        aps["wp"], aps["masks"], aps["outp"], aps["xT"],
    )

    with ExitStack() as ctx:
        const = ctx.enter_context(tc.tile_pool(name="const", bufs=1))
        big = ctx.enter_context(tc.tile_pool(name="big", bufs=1))
        wq_pool = ctx.enter_context(tc.tile_pool(name="wq", bufs=8))
        es_pool = ctx.enter_context(tc.tile_pool(name="es", bufs=3))
        ytok_pool = ctx.enter_context(tc.tile_pool(name="ytok", bufs=2))
        nrm_pool = ctx.enter_context(tc.tile_pool(name="nrm", bufs=4))
        ostg = ctx.enter_context(tc.tile_pool(name="ostg", bufs=4))
        psA = ctx.enter_context(tc.tile_pool(name="psA", bufs=2, space="PSUM"))
        psS = ctx.enter_context(tc.tile_pool(name="psS", bufs=4, space="PSUM"))
        psYp = ctx.enter_context(tc.tile_pool(name="psY", bufs=1, space="PSUM"))

        masks_sb = const.tile([P, P], bf16)
        nc.sync.dma_start(masks_sb[:], masks[:])
        bqk_sb = const.tile([P, JQK], f32)
        nc.sync.dma_start(bqk_sb[:], bqk[:])
        bv_rep = const.tile([P, CL], f32)
        nc.sync.dma_start(bv_rep[:], bv[None, :].to_broadcast([P, CL]))

        xT_sb = big.tile([P, KC, T], bf16)
        qkT = big.tile([P, JQK, T], bf16)
        vaug = big.tile([P, NJC, HL, D + 1], bf16)
        yT = big.tile([P, 4, NJC, P], bf16)
        wv_sb = big.tile([P, KC, CL], bf16)
        wp_sb = big.tile([P, 4, C], bf16)
        psY0 = psYp.tile([P, 4, D + 1], f32, tag="y0")
        psY1 = psYp.tile([P, 4, D + 1], f32, tag="y1")
        psY2 = [psY0, psY1]

        nc.gpsimd.memset(vaug[:, :, :, D : D + 1], 1.0)
        nc.vector.memset(psY0[:], 0.0)
        nc.vector.memset(psY1[:], 0.0)

        # ---------------- filler queue (keyed; need() force-drains) --------
        fillers = deque()
        emitted = set()
        _rate = {"acc": 0.0}

        def run_next():
            key, thunk = fillers.popleft()
            thunk()
            emitted.add(key)

        def need(key):
            while fillers and key not in emitted:
                run_next()

        def drain(frac):
            _rate["acc"] += frac
            while fillers and _rate["acc"] >= 1.0:
                _rate["acc"] -= 1.0
                run_next()

        # ---------------- qkv / projection producers -----------------------
        wq_box = {}

        def qk_chunk(j, tci):
            if j not in wq_box:
                t = wq_pool.tile([P, KC, P], bf16, tag="wq", name=f"wq{j}")
                nc.sync.dma_start(t[:], wqk[j])
                wq_box[j] = t
            wq_t = wq_box[j]
            ps = psA.tile([P, 512], f32, tag="a")
            for k in range(KC):
                nc.tensor.matmul(
                    ps[:], wq_t[:, k, :],
                    xT_sb[:, k, tci * 512 : tci * 512 + 512],
                    start=(k == 0), stop=(k == KC - 1),
                )
            nc.vector.tensor_scalar(
                out=qkT[:, j, tci * 512 : tci * 512 + 512], in0=ps[:],
                scalar1=bqk_sb[:, j : j + 1], scalar2=None, op0=ADD,
            )

        def v_chunk(jc):
            ps = psA.tile([P, 512], f32, tag="a")
            for k in range(KC):
                nc.tensor.matmul(
                    ps[:], xT_sb[:, k, jc * P : (jc + 1) * P], wv_sb[:, k, :],
                    start=(k == 0), stop=(k == KC - 1),
                )
            nc.vector.tensor_tensor(
                vaug[:, jc, :, 0:D],
                ps[:].rearrange("p (h d) -> p h d", h=HL),
                bv_rep[:].rearrange("p (h d) -> p h d", h=HL), ADD,
            )

        def c_tile(tcb, oc):
            ps = psA.tile([P, 512], f32, tag="a")
            for li in range(4):
                nc.tensor.matmul(
                    ps[:], yT[:, li, tcb, :],
                    wp_sb[:, li, oc * 512 : oc * 512 + 512],
                    start=(li == 0), stop=(li == 3),
                )
            ot = ostg.tile([P, 512], f32, tag="o")
            if tcb >= 12:
                nc.scalar.copy(ot[:], ps[:])
            else:
                nc.vector.tensor_copy(ot[:], ps[:])
            eng = nc.sync if (tcb + oc) % 2 == 0 else nc.gpsimd
            eng.dma_start(
                outp[tcb * P : (tcb + 1) * P, oc * 512 : oc * 512 + 512], ot[:]
            )

        # ---------------- attention pass: (head, 512-query block) ---------
        pass_idx = [0]
        ytok_box = {}

        def emit_pass(h, qb):
            pr, half = h // 2, h % 2
            base = half * 64
            qTh = qkT[base : base + 64, pr, :]
            kTh = qkT[base : base + 64, 4 + pr, :]
            lo = qb * 512
            nst = 4 * qb + 4
            psY = psY2[pass_idx[0] % 2]
            pass_idx[0] += 1
            es_tiles = {}

            def emit_strip(jc):
                need(("qk", pr, qb))
                need(("qk", 4 + pr, jc // 4))
                need(("v", jc))
                drain(0.18)
                start_q = max(jc * P, lo)
                w = lo + 512 - start_q
                ps = psS.tile([P, 512], f32, tag="s")
                nc.tensor.matmul(
                    ps[:, 0:w], kTh[:, jc * P : (jc + 1) * P],
                    qTh[:, start_q : lo + 512], start=True, stop=True,
                )
                es_t = es_pool.tile([P, 512], bf16, tag="es")
                nc.scalar.activation(es_t[:, :w], ps[:, 0:w], EXP, scale=0.125)
                if jc * P >= lo:
                    nc.vector.tensor_tensor(
                        es_t[:, 0:P], es_t[:, 0:P], masks_sb[:], MUL
                    )
                es_tiles[jc] = (es_t, start_q)

            def emit_av(jc):
                es_t, start_q = es_tiles.pop(jc)
                for qc in range(start_q // P, 4 * qb + 4):
                    c = qc - 4 * qb
                    col = qc * P - start_q
                    # psY is zeroed explicitly (memset below): start=True
                    # would wipe the whole bank-wide zero region shared by
                    # the sibling chunk accumulators.
                    nc.tensor.matmul(
                        psY[:, c, :], es_t[:, col : col + P],
                        vaug[:, jc, h, :],
                        start=False, stop=False, skip_group_check=True,
                    )

            emit_strip(0)
            for jc in range(1, nst):
                emit_strip(jc)
                emit_av(jc - 1)
            emit_av(nst - 1)

            # batched normalize + transpose + re-zero; runs on DVE while the
            # next pass accumulates into the other psY tile (bank-disjoint).
            # The two heads of a pair write the two 64-col halves of one
            # y_tok2 tile; after the odd head, plain 2D [128,128] transposes
            # land both halves as yT's (h%2)*64+d row layout in one shot.
            # (3D/multi-column-tile DMA transposes diverge from CoreSim's
            # model on the device -- only the 2D 128x128 form is safe.)
            rec = nrm_pool.tile([P, 4], f32, tag="rec")
            nc.vector.reciprocal(rec[:], psY[:, :, D])
            if half == 0:
                ytok_box[pr] = ytok_pool.tile(
                    [P, 4, P], bf16, tag="yt", name=f"yt{pr}_{qb}"
                )
            yt = ytok_box[pr]
            for c in range(4):
                nc.vector.tensor_scalar(
                    out=yt[:, c, base : base + 64], in0=psY[:, c, 0:D],
                    scalar1=rec[:, c : c + 1], scalar2=None, op0=MUL,
                )
            if half == 1:
                for c in range(4):
                    nc.sync.dma_start_transpose(
                        yT[:, pr, 4 * qb + c, :], yt[:, c, :]
                    )
            nc.vector.memset(psY[:], 0.0)

        # ---------------- emission ----------------------------------------
        qorder = [("qk", 0, 0), ("qk", 4, 0)]
        qorder += [("v", i) for i in range(4)]
        qorder += [("qk", 1, 0), ("qk", 5, 0), ("qk", 2, 0), ("qk", 6, 0),
                   ("qk", 3, 0), ("qk", 7, 0)]
        qorder += [("qk", 0, 1), ("qk", 4, 1)]
        qorder += [("v", i) for i in range(4, 8)]
        qorder += [("qk", 1, 1), ("qk", 5, 1), ("qk", 2, 1), ("qk", 6, 1),
                   ("qk", 3, 1), ("qk", 7, 1)]
        qorder += [("proj", tcb, oc) for tcb in range(0, 4) for oc in (0, 1)]
        qorder += [("qk", 0, 2), ("qk", 4, 2)]
        qorder += [("v", i) for i in range(8, 12)]
        qorder += [("qk", 1, 2), ("qk", 5, 2), ("qk", 2, 2), ("qk", 6, 2),
                   ("qk", 3, 2), ("qk", 7, 2)]
        qorder += [("proj", tcb, oc) for tcb in range(4, 8) for oc in (0, 1)]
        qorder += [("qk", 0, 3), ("qk", 4, 3)]
        qorder += [("v", i) for i in range(12, 16)]
        qorder += [("qk", 1, 3), ("qk", 5, 3), ("qk", 2, 3), ("qk", 6, 3),
                   ("qk", 3, 3), ("qk", 7, 3)]
        qorder += [("proj", tcb, oc) for tcb in range(8, 12) for oc in (0, 1)]

        def make_thunk(key):
            kind = key[0]
            if kind == "qk":
                return lambda: qk_chunk(key[1], key[2])
            if kind == "v":
                return lambda: v_chunk(key[1])
            return lambda: c_tile(key[1], key[2])

        for key in qorder:
            fillers.append((key, make_thunk(key)))

        # prelude: xT loads interleaved with first qk chunks so PE starts
        # early; weights stream behind
        for k in range(KC):
            nc.sync.dma_start(xT_sb[:, k, 0:512], xT[:, k, 0:512])
        need(("qk", 0, 0))
        nc.sync.dma_start(wv_sb[:], wv[:])
        need(("qk", 4, 0))
        nc.sync.dma_start(xT_sb[:, :, 512:1024], xT[:, :, 512:1024])
        need(("v", 1))
        nc.sync.dma_start(xT_sb[:, :, 1024:1536], xT[:, :, 1024:1536])
        nc.sync.dma_start(xT_sb[:, :, 1536:2048], xT[:, :, 1536:2048])
        nc.sync.dma_start(wp_sb[:], wp[:])

        for qb in range(NQB):
            for h in range(HL):
                emit_pass(h, qb)

        while fillers:
            run_next()
        for tcb in range(12, NJC):
            for oc in range(2):
                c_tile(tcb, oc)


_CACHE = {}


def build_nc():
    if "nc" in _CACHE:
        return _CACHE["nc"]
    nc = bacc.Bacc(
        "TRN2",
        target_bir_lowering=False,
        debug=False,
        enable_asserts=False,
        num_devices=8,
    )
    aps = {
        "wqk": nc.dram_tensor("wqk", [JQK, P, KC, P], bf16, kind="ExternalInput").ap(),
        "bqk": nc.dram_tensor("bqk", [P, JQK], f32, kind="ExternalInput").ap(),
        "wv": nc.dram_tensor("wv", [P, KC, CL], bf16, kind="ExternalInput").ap(),
        "bv": nc.dram_tensor("bv", [CL], f32, kind="ExternalInput").ap(),
        "wp": nc.dram_tensor("wp", [P, 4, C], bf16, kind="ExternalInput").ap(),
        "masks": nc.dram_tensor("masks", [P, P], bf16, kind="ExternalInput").ap(),
        "xT": nc.dram_tensor("xT", [P, KC, T], bf16, kind="ExternalInput").ap(),
        "outp": nc.dram_tensor("outp", [T, C], f32, kind="ExternalOutput").ap(),
    }
    with tile.TileContext(nc) as tc:
        build_body(tc, aps)
    nc.compile()
    _CACHE["nc"] = nc
    return nc


def make_in_maps(x, w_attn, b_attn, w_proj, b_proj):
    masks = np.triu(np.ones((P, P), dtype=np.float32)).astype(ml_dtypes.bfloat16)
    in_maps = []
    for core in range(8):
        b, g = core // 2, core % 2
        xT_b = np.ascontiguousarray(
            x[b].T.reshape(KC, P, T).transpose(1, 0, 2)
        ).astype(ml_dtypes.bfloat16)
        qcols = slice(g * CL, (g + 1) * CL)
        kcols = slice(C + g * CL, C + (g + 1) * CL)
        vcols = slice(2 * C + g * CL, 2 * C + (g + 1) * CL)
        wqk_b = np.ascontiguousarray(
            np.concatenate([w_attn[:, qcols], w_attn[:, kcols]], axis=1)
            .astype(ml_dtypes.bfloat16)
            .reshape(KC, P, JQK, P)
            .transpose(2, 1, 0, 3)
        )
        bqk_b = np.ascontiguousarray(
            np.concatenate([b_attn[qcols], b_attn[kcols]]).reshape(JQK, P).T
        )
        wv_b = np.ascontiguousarray(
            w_attn[:, vcols].reshape(KC, P, CL).transpose(1, 0, 2)
        ).astype(ml_dtypes.bfloat16)
        wp_b = np.ascontiguousarray(
            w_proj[g * CL : (g + 1) * CL, :].reshape(4, P, C).transpose(1, 0, 2)
        ).astype(ml_dtypes.bfloat16)
        in_maps.append(
            {
                "wqk": wqk_b,
                "bqk": bqk_b,
                "wv": wv_b,
                "bv": np.ascontiguousarray(b_attn[vcols]),
                "wp": wp_b,
                "masks": masks,
                "xT": xT_b,
            }
        )
    return in_maps


def combine(parts, b_proj):
    return np.stack(
        [parts[2 * b] + parts[2 * b + 1] + b_proj[None, :] for b in range(B)]
    ).astype(np.float32)


def kernel(x, w_attn, b_attn, w_proj, b_proj, _trace=False, **run_kwargs):
    x = np.asarray(x, dtype=np.float32)
    w_attn = np.asarray(w_attn, dtype=np.float32)
    b_attn = np.asarray(b_attn, dtype=np.float32)
    w_proj = np.asarray(w_proj, dtype=np.float32)
    b_proj = np.asarray(b_proj, dtype=np.float32)

    nc = build_nc()
    in_maps = make_in_maps(x, w_attn, b_attn, w_proj, b_proj)
    try:
        res = bass_utils.run_bass_kernel_spmd(
            nc, in_maps, core_ids=list(range(8)), trace=_trace, **run_kwargs
        )
    except Exception:
        # transient NRT device wedge: one retry
        res = bass_utils.run_bass_kernel_spmd(
            nc, in_maps, core_ids=list(range(8)), trace=_trace, **run_kwargs
        )
    parts = [res.results[i]["outp"] for i in range(8)]
    out = combine(parts, b_proj)
    if _trace:
        return out, res
    return out
